# revision 20
# baseline (speedup 1.0000x reference)
"""LSTM carry kernel for trn2: B=8192, T=1024, D=H=16, out = softmax(c_T @ Wd + bd).

Data-parallel over 8 cores (1024 batch rows each). Within a core, the batch is
split as b = 128*m + j (m in 0..7, j in 0..127), blocks m = 2*(2c+p) + mu with
chain c in {0,1}, p in {0,1}, mu in {0,1}. Two independent recurrence chains
(c=0: blocks 0-3, c=1: blocks 4-7) pipeline against each other.

Everything stays in K-major ("z^T") form [(gate4, mu2, h16)=128p, (p2, j128)=256f]
so the elementwise cell update emits h~^T directly as the next matmul's moving
operand - no transposes, no PSUM->SBUF copies.

Per chain per step:
  z   = matmul(lhsT=WLH[65,128] const, rhs=[xT(32) | h~T(32) | ones(1)][65,256])
        (weights stationary; bias rides the ones row; start=True)   [PE]
  bt  = sigmoid(z)  -> fp16, gates i@0:32 f@32:64 g@64:96 o@96:128  [ACT]
  q   = (bt_g - 0.5) * bt_i                (= sig_i * tanh(zg) / 2) [DVE stt]
  v   = bt_f * Sig_prev                    (Sig = 2c, fp16 state)   [DVE tt]
  Sig = 4*q + v                                                     [DVE stt]
  sS  = sigmoid(Sig)                       (= (tanh(c')+1)/2)       [ACT]
  h~T = (sS - 0.5) * bt_o  -> arena rows 32:64 of slot t+1          [DVE stt]

Math: state Sig = 2*cell; tanh(u) = 2*sigmoid(2u)-1; g-columns of weights
pre-scaled x2, Wh pre-scaled x2 (h~ = h/2); head uses c = Sig/2 with a 2-class
softmax computed exactly as p0 = sigmoid(c @ (Wd0-Wd1) + bd0-bd1), p1 = 1-p0.
"""
import sys
sys.path.insert(0, "/opt/trn_rl_repo")
import numpy as np

B, T, D, H = 8192, 1024, 16, 16
NCORES = 8
BS = B // NCORES          # 1024 rows per core
NCH = 2                   # recurrence chains per core


def _build(nsteps: int):
    import concourse.bass as bass
    import concourse.bacc as bacc
    import concourse.mybir as mybir
    from concourse import tile

    assert nsteps % 8 == 0
    NW = nsteps // 8

    f32 = mybir.dt.float32
    f16 = mybir.dt.float16
    A = mybir.AluOpType
    AF = mybir.ActivationFunctionType

    nc = bacc.Bacc("TRN2", target_bir_lowering=False, debug=False)
    tc = tile.TileContext(nc)

    xt_d = nc.dram_tensor("xt", [NCH, NW, 32, 8, 256], f16, kind="ExternalInput").ap()
    wlh_d = nc.dram_tensor("wlh", [65, 128], f16, kind="ExternalInput").ap()
    wdh_d = nc.dram_tensor("wdh", [32, 2], f16, kind="ExternalInput").ap()
    bdh_d = nc.dram_tensor("bdh", [2, 1], f32, kind="ExternalInput").ap()
    out_d = nc.dram_tensor("out", [BS, 2], f32, kind="ExternalOutput").ap()

    with tc, tc.tile_pool(name="const", bufs=1) as cpool, \
         tc.tile_pool(name="state", bufs=1) as stpool, \
         tc.tile_pool(name="bt", bufs=3) as btpool, \
         tc.tile_pool(name="cell", bufs=2) as cellpool, \
         tc.tile_pool(name="psz", bufs=2, space="PSUM") as pz, \
         tc.tile_pool(name="pshead", bufs=1, space="PSUM") as ph:

        wlh = cpool.tile([128, 128], f16, tag="wlh")
        wdh = cpool.tile([64, 2], f16, tag="wdh")
        bdh = cpool.tile([2, 1], f32, tag="bdh")
        nc.sync.dma_start(wlh[0:65, :], wlh_d)
        nc.sync.dma_start(wdh[32:64, :], wdh_d)
        nc.sync.dma_start(bdh[:], bdh_d)

        # arenas: [65p used, 8 slots, 256] rows 0:32 = xT (DMA), 32:64 = h~T,
        # row 64 = ones (bias row). Two buffers per chain (window ping-pong).
        arena = [[cpool.tile([128, 8, 256], f16, tag=f"ar{c}{bf}", name=f"ar{c}{bf}")
                  for bf in range(2)] for c in range(NCH)]
        # Sig state (= 2*cell), fp16, two parities per chain. Lives at
        # partition base 32 (colocated with the f gate for the v product).
        Sig = [[stpool.tile([64, 256], f16, tag=f"Sig{c}{par}", name=f"Sig{c}{par}")
                for par in range(2)] for c in range(NCH)]
        # tanh(cell) lives at base 96 (colocated with the o gate).
        sSt = [stpool.tile([128, 256], f16, tag=f"sS{c}", name=f"sS{c}")
               for c in range(NCH)]

        for c in range(NCH):
            for bf in range(2):
                nc.vector.memset(arena[c][bf][64:65, :, :], 1.0)
            nc.vector.memset(arena[c][0][32:64, 0:1, :], 0.0)   # h~(-1) = 0
            nc.vector.memset(Sig[c][1][32:64, :], 0.0)          # Sig(-1) = 0

            nc.sync.dma_start(arena[c][0][0:32, :, :], xt_d[c, 0])

        for t in range(nsteps):
            s, w, par = t % 8, t // 8, t % 2
            bf = w % 2
            last = t == nsteps - 1

            order = (0, 1) if t % 2 == 0 else (1, 0)
            if s == 0 and w + 1 < NW:
                for c in order:
                    nc.sync.dma_start(arena[c][(w + 1) % 2][0:32, :, :],
                                      xt_d[c, w + 1])

            zt, bt = {}, {}
            for c in order:
                z = pz.tile([128, 256], f32, tag=f"z{c}", name=f"z{c}")
                nc.tensor.matmul(z[:], lhsT=wlh[0:65, :],
                                 rhs=arena[c][bf][0:65, s:s + 1, :],
                                 start=True, stop=True)
                zt[c] = z
            for c in order:
                b_ = btpool.tile([128, 256], f16, tag=f"bt{c}", name=f"bt{c}")
                nc.scalar.activation(b_[:], zt[c][:], AF.Sigmoid)
                bt[c] = b_
            # cell update per chain: gsh = 4*G-2 (relocate g to base 0);
            # qh = gsh*sig_i; v = sig_f*Sig_prev; Sig = qh+v; sS = sigmoid(Sig)
            # plain TT (2x dve mode) beats STT (1x) on hardware
            for c in order:
                gsh = cellpool.tile([32, 256], f16, tag=f"gsh{c}", name=f"gsh{c}")
                nc.vector.tensor_scalar(gsh[:], bt[c][64:96, :], 4.0, -2.0,
                                        op0=A.mult, op1=A.add)
                v_ = cellpool.tile([32, 256], f16, tag=f"v{c}", name=f"v{c}")
                nc.vector.tensor_tensor(v_[:], bt[c][32:64, :],
                                        Sig[c][1 - par][32:64, :], op=A.mult)
                q_ = cellpool.tile([32, 256], f16, tag=f"q{c}", name=f"q{c}")
                nc.vector.tensor_tensor(q_[:], gsh[:], bt[c][0:32, :], op=A.mult)
                nc.vector.tensor_tensor(Sig[c][par][32:64, :], q_[:], v_[:],
                                        op=A.add)
            if not last:
                # th = tanh(Sig/2) = tanh(cell); h = th * sig_o goes straight
                # into the next step's lhsT rows 32:64 (plain TT, 2x mode)
                for c in order:
                    nc.scalar.activation(sSt[c][96:128, :], Sig[c][par][32:64, :],
                                         AF.Tanh, scale=0.5)
                s2, bf2 = (t + 1) % 8, ((t + 1) // 8) % 2
                for c in order:
                    nc.vector.tensor_tensor(
                        arena[c][bf2][32:64, s2:s2 + 1, :], sSt[c][96:128, :],
                        bt[c][96:128, :], op=A.mult)

        # ---- output head: p0 = sigmoid((Sig/2)@dWd + dbd), p1 = 1 - p0 ----
        fpar = (nsteps - 1) % 2
        for c in range(NCH):
            hp = ph.tile([2, 256], f32, tag=f"hp{c}", name=f"hp{c}")
            nc.tensor.matmul(hp[:], lhsT=wdh[32:64, 0:2],
                             rhs=Sig[c][fpar][32:64, :],
                             start=True, stop=True)
            p0 = btpool.tile([2, 256], f32, tag=f"p0{c}", name=f"p0{c}")
            p1 = btpool.tile([2, 256], f32, tag=f"p1{c}", name=f"p1{c}")
            nc.scalar.activation(p0[:], hp[:], AF.Sigmoid, bias=bdh[:])
            nc.vector.tensor_scalar(p1[:], p0[:], -1.0, 1.0,
                                    op0=A.mult, op1=A.add)
            for mu in range(2):
                for p in range(2):
                    m = 2 * (2 * c + p) + mu
                    nc.sync.dma_start(out=out_d[128 * m:128 * m + 128, 0:1],
                                      in_=p0[mu:mu + 1, 128 * p:128 * p + 128])
                    nc.sync.dma_start(out=out_d[128 * m:128 * m + 128, 1:2],
                                      in_=p1[mu:mu + 1, 128 * p:128 * p + 128])

    nc.finalize()
    return nc


def _prep_params(Wi, Wh, b, Wd, bd):
    Wi = np.asarray(Wi, np.float32)
    Wh = np.asarray(Wh, np.float32)
    b = np.asarray(b, np.float32)
    Wd = np.asarray(Wd, np.float32)
    bd = np.asarray(bd, np.float32)

    Wip = Wi.copy()
    Whp = Wh.copy()                 # arena h rows hold h = tanh(c)*sig_o
    bp = b.copy()
    Wip[:, 32:48] *= 2.0            # tanh-via-sigmoid on the g gate
    Whp[:, 32:48] *= 2.0
    bp[32:48] *= 2.0

    # wlh[k, 32*gt + 16*mu + h']: k=16*mu+d -> Wip[d, 16*gt+h'] (mu diag)
    #                             k=32+16*mu+h -> Whp[h, 16*gt+h'] (mu diag)
    #                             k=64 -> bp[16*gt+h']
    wlh = np.zeros((65, 4, 2, 16), np.float32)
    Wiv = Wip.reshape(16, 4, 16)
    Whv = Whp.reshape(16, 4, 16)
    for mu in range(2):
        wlh[16 * mu:16 * mu + 16, :, mu, :] = Wiv
        wlh[32 + 16 * mu:48 + 16 * mu, :, mu, :] = Whv
    wlh[64, :, 0, :] = bp.reshape(4, 16)
    wlh[64, :, 1, :] = bp.reshape(4, 16)
    wlh = wlh.reshape(65, 128).astype(np.float16)

    wdh = np.zeros((32, 2), np.float32)
    dwd = 0.5 * (Wd[:, 0] - Wd[:, 1])   # c = Sig/2
    for mu in range(2):
        wdh[16 * mu:16 * mu + 16, mu] = dwd
    wdh = wdh.astype(np.float16)
    bdh = np.full((2, 1), float(bd[0] - bd[1]), np.float32)
    return wlh, wdh, bdh


def _prep_x(x, nsteps):
    # per core -> [chain2, NW, 32=(mu,d), 8=slot, 256=(p^,j)] fp16
    xs = np.asarray(x[:, :nsteps, :], np.float16)
    outs = []
    for cid in range(NCORES):
        a = xs[cid * BS:(cid + 1) * BS]               # [1024, nsteps, 16]
        a = a.reshape(8, 128, nsteps // 8, 8, 16)     # [m, j, w, s, d]
        a = a.reshape(2, 2, 2, 128, nsteps // 8, 8, 16)  # [c, p^, mu, j, w, s, d]
        a = a.transpose(0, 4, 2, 6, 5, 1, 3)          # [c, w, mu, d, s, p^, j]
        outs.append(np.ascontiguousarray(
            a.reshape(2, nsteps // 8, 32, 8, 256)))
    return outs


_CACHE = {}


def kernel(x, Wi, Wh, b, Wd, bd, nsteps=T, _profile=False):
    from concourse import bass_utils
    wlh, wdh, bdh = _prep_params(Wi, Wh, b, Wd, bd)
    xts = _prep_x(np.asarray(x, np.float32), nsteps)

    if nsteps not in _CACHE:
        _CACHE[nsteps] = _build(nsteps)
    nc = _CACHE[nsteps]

    in_maps = [dict(xt=xts[cid], wlh=wlh, wdh=wdh, bdh=bdh)
               for cid in range(NCORES)]
    res = bass_utils.run_bass_kernel_spmd(nc, in_maps, core_ids=list(range(NCORES)),
                                          trace=_profile)
    outs = [r["out"] for r in res.results]
    full = np.concatenate(outs, 0)
    if _profile:
        return full, res
    return full


# revision 22
# speedup vs baseline: 1.0691x; 1.0691x over previous
"""LSTM carry kernel for trn2: B=8192, T=1024, D=H=16, out = softmax(c_T @ Wd + bd).

Data-parallel over 8 cores (1024 batch rows each). Within a core, the batch is
split as b = 128*m + j (m in 0..7, j in 0..127), blocks m = 2*(2c+p) + mu with
chain c in {0,1}, p in {0,1}, mu in {0,1}. Two independent recurrence chains
(c=0: blocks 0-3, c=1: blocks 4-7) pipeline against each other.

Everything stays in K-major ("z^T") form [(gate4, mu2, h16)=128p, (p2, j128)=256f]
so the elementwise cell update emits h~^T directly as the next matmul's moving
operand - no transposes, no PSUM->SBUF copies.

Per chain per step:
  z   = matmul(lhsT=WLH[65,128] const, rhs=[xT(32) | h~T(32) | ones(1)][65,256])
        (weights stationary; bias rides the ones row; start=True)   [PE]
  bt  = sigmoid(z)  -> fp16, gates i@0:32 f@32:64 g@64:96 o@96:128  [ACT]
  q   = (bt_g - 0.5) * bt_i                (= sig_i * tanh(zg) / 2) [DVE stt]
  v   = bt_f * Sig_prev                    (Sig = 2c, fp16 state)   [DVE tt]
  Sig = 4*q + v                                                     [DVE stt]
  sS  = sigmoid(Sig)                       (= (tanh(c')+1)/2)       [ACT]
  h~T = (sS - 0.5) * bt_o  -> arena rows 32:64 of slot t+1          [DVE stt]

Math: state Sig = 2*cell; tanh(u) = 2*sigmoid(2u)-1; g-columns of weights
pre-scaled x2, Wh pre-scaled x2 (h~ = h/2); head uses c = Sig/2 with a 2-class
softmax computed exactly as p0 = sigmoid(c @ (Wd0-Wd1) + bd0-bd1), p1 = 1-p0.
"""
import sys
sys.path.insert(0, "/opt/trn_rl_repo")
import numpy as np

B, T, D, H = 8192, 1024, 16, 16
NCORES = 8
BS = B // NCORES          # 1024 rows per core
NCH = 2                   # recurrence chains per core


def _build(nsteps: int):
    import concourse.bass as bass
    import concourse.bacc as bacc
    import concourse.mybir as mybir
    from concourse import tile

    assert nsteps % 8 == 0
    NW = nsteps // 8

    f32 = mybir.dt.float32
    f16 = mybir.dt.float16
    A = mybir.AluOpType
    AF = mybir.ActivationFunctionType

    nc = bacc.Bacc("TRN2", target_bir_lowering=False, debug=False)
    tc = tile.TileContext(nc)

    xt_d = nc.dram_tensor("xt", [NCH, NW, 32, 8, 256], f16, kind="ExternalInput").ap()
    wlh_d = nc.dram_tensor("wlh", [65, 128], f16, kind="ExternalInput").ap()
    wdh_d = nc.dram_tensor("wdh", [32, 2], f16, kind="ExternalInput").ap()
    bdh_d = nc.dram_tensor("bdh", [2, 1], f32, kind="ExternalInput").ap()
    out_d = nc.dram_tensor("out", [BS, 2], f32, kind="ExternalOutput").ap()

    with tc, tc.tile_pool(name="const", bufs=1) as cpool, \
         tc.tile_pool(name="state", bufs=1) as stpool, \
         tc.tile_pool(name="bt", bufs=3) as btpool, \
         tc.tile_pool(name="cell", bufs=2) as cellpool, \
         tc.tile_pool(name="psz", bufs=2, space="PSUM") as pz, \
         tc.tile_pool(name="pshead", bufs=1, space="PSUM") as ph:

        wlh = cpool.tile([128, 128], f16, tag="wlh")
        wdh = cpool.tile([64, 2], f16, tag="wdh")
        bdh = cpool.tile([2, 1], f32, tag="bdh")
        nc.sync.dma_start(wlh[0:65, :], wlh_d)
        nc.sync.dma_start(wdh[32:64, :], wdh_d)
        nc.sync.dma_start(bdh[:], bdh_d)

        # arenas: [65p used, 8 slots, 256] rows 0:32 = xT (DMA), 32:64 = h~T,
        # row 64 = ones (bias row). Two buffers per chain (window ping-pong).
        arena = [[cpool.tile([128, 8, 256], f16, tag=f"ar{c}{bf}", name=f"ar{c}{bf}")
                  for bf in range(2)] for c in range(NCH)]
        # Sig state (= 2*cell), fp16, two parities per chain. Lives at
        # partition base 32 (colocated with the f gate for the v product).
        Sig = [[stpool.tile([64, 256], f16, tag=f"Sig{c}{par}", name=f"Sig{c}{par}")
                for par in range(2)] for c in range(NCH)]
        # tanh(cell) lives at base 96 (colocated with the o gate).
        sSt = [stpool.tile([128, 256], f16, tag=f"sS{c}", name=f"sS{c}")
               for c in range(NCH)]
        junk = ph.tile([128, 512], f32, tag="junk")   # clock warmer target

        for c in range(NCH):
            for bf in range(2):
                nc.vector.memset(arena[c][bf][64:65, :, :], 1.0)
            nc.vector.memset(arena[c][0][32:64, 0:1, :], 0.0)   # h~(-1) = 0
            nc.vector.memset(Sig[c][1][32:64, :], 0.0)          # Sig(-1) = 0

            nc.sync.dma_start(arena[c][0][0:32, :, :], xt_d[c, 0])

        for t in range(nsteps):
            s, w, par = t % 8, t // 8, t % 2
            bf = w % 2
            last = t == nsteps - 1

            order = (0, 1) if t % 2 == 0 else (1, 0)
            if s == 0 and w + 1 < NW:
                for c in order:
                    nc.sync.dma_start(arena[c][(w + 1) % 2][0:32, :, :],
                                      xt_d[c, w + 1])

            zt, bt = {}, {}
            for c in order:
                z = pz.tile([128, 256], f32, tag=f"z{c}", name=f"z{c}")
                nc.tensor.matmul(z[:], lhsT=wlh[0:65, :],
                                 rhs=arena[c][bf][0:65, s:s + 1, :],
                                 start=True, stop=True)
                zt[c] = z
            for c in order:
                b_ = btpool.tile([128, 256], f16, tag=f"bt{c}", name=f"bt{c}")
                nc.scalar.activation(b_[:], zt[c][:], AF.Sigmoid)
                bt[c] = b_
            # cell update per chain: gsh = 4*G-2 (relocate g to base 0);
            # qh = gsh*sig_i; v = sig_f*Sig_prev; Sig = qh+v; sS = sigmoid(Sig)
            # plain TT (2x dve mode) beats STT (1x) on hardware
            for c in order:
                gsh = cellpool.tile([32, 256], f16, tag=f"gsh{c}", name=f"gsh{c}")
                nc.vector.tensor_scalar(gsh[:], bt[c][64:96, :], 4.0, -2.0,
                                        op0=A.mult, op1=A.add)
                v_ = cellpool.tile([32, 256], f16, tag=f"v{c}", name=f"v{c}")
                nc.vector.tensor_tensor(v_[:], bt[c][32:64, :],
                                        Sig[c][1 - par][32:64, :], op=A.mult)
                q_ = cellpool.tile([32, 256], f16, tag=f"q{c}", name=f"q{c}")
                nc.vector.tensor_tensor(q_[:], gsh[:], bt[c][0:32, :], op=A.mult)
                nc.vector.tensor_tensor(Sig[c][par][32:64, :], q_[:], v_[:],
                                        op=A.add)
            if not last:
                # th = tanh(Sig/2) = tanh(cell); h = th * sig_o goes straight
                # into the next step's lhsT rows 32:64 (plain TT, 2x mode)
                for c in order:
                    nc.scalar.activation(sSt[c][96:128, :], Sig[c][par][32:64, :],
                                         AF.Tanh, scale=0.5)
                s2, bf2 = (t + 1) % 8, ((t + 1) // 8) % 2
                for c in order:
                    nc.vector.tensor_tensor(
                        arena[c][bf2][32:64, s2:s2 + 1, :], sSt[c][96:128, :],
                        bt[c][96:128, :], op=A.mult)
                # junk matmul keeps the clock governor boosted
                nc.tensor.matmul(junk[:], lhsT=wlh[0:65, :],
                                 rhs=arena[0][bf][0:65, 0:2, :],
                                 start=True, stop=True)

        # ---- output head: p0 = sigmoid((Sig/2)@dWd + dbd), p1 = 1 - p0 ----
        fpar = (nsteps - 1) % 2
        for c in range(NCH):
            hp = ph.tile([2, 256], f32, tag=f"hp{c}", name=f"hp{c}")
            nc.tensor.matmul(hp[:], lhsT=wdh[32:64, 0:2],
                             rhs=Sig[c][fpar][32:64, :],
                             start=True, stop=True)
            p0 = btpool.tile([2, 256], f32, tag=f"p0{c}", name=f"p0{c}")
            p1 = btpool.tile([2, 256], f32, tag=f"p1{c}", name=f"p1{c}")
            nc.scalar.activation(p0[:], hp[:], AF.Sigmoid, bias=bdh[:])
            nc.vector.tensor_scalar(p1[:], p0[:], -1.0, 1.0,
                                    op0=A.mult, op1=A.add)
            for mu in range(2):
                for p in range(2):
                    m = 2 * (2 * c + p) + mu
                    nc.sync.dma_start(out=out_d[128 * m:128 * m + 128, 0:1],
                                      in_=p0[mu:mu + 1, 128 * p:128 * p + 128])
                    nc.sync.dma_start(out=out_d[128 * m:128 * m + 128, 1:2],
                                      in_=p1[mu:mu + 1, 128 * p:128 * p + 128])

    nc.finalize()
    return nc


def _prep_params(Wi, Wh, b, Wd, bd):
    Wi = np.asarray(Wi, np.float32)
    Wh = np.asarray(Wh, np.float32)
    b = np.asarray(b, np.float32)
    Wd = np.asarray(Wd, np.float32)
    bd = np.asarray(bd, np.float32)

    Wip = Wi.copy()
    Whp = Wh.copy()                 # arena h rows hold h = tanh(c)*sig_o
    bp = b.copy()
    Wip[:, 32:48] *= 2.0            # tanh-via-sigmoid on the g gate
    Whp[:, 32:48] *= 2.0
    bp[32:48] *= 2.0

    # wlh[k, 32*gt + 16*mu + h']: k=16*mu+d -> Wip[d, 16*gt+h'] (mu diag)
    #                             k=32+16*mu+h -> Whp[h, 16*gt+h'] (mu diag)
    #                             k=64 -> bp[16*gt+h']
    wlh = np.zeros((65, 4, 2, 16), np.float32)
    Wiv = Wip.reshape(16, 4, 16)
    Whv = Whp.reshape(16, 4, 16)
    for mu in range(2):
        wlh[16 * mu:16 * mu + 16, :, mu, :] = Wiv
        wlh[32 + 16 * mu:48 + 16 * mu, :, mu, :] = Whv
    wlh[64, :, 0, :] = bp.reshape(4, 16)
    wlh[64, :, 1, :] = bp.reshape(4, 16)
    wlh = wlh.reshape(65, 128).astype(np.float16)

    wdh = np.zeros((32, 2), np.float32)
    dwd = 0.5 * (Wd[:, 0] - Wd[:, 1])   # c = Sig/2
    for mu in range(2):
        wdh[16 * mu:16 * mu + 16, mu] = dwd
    wdh = wdh.astype(np.float16)
    bdh = np.full((2, 1), float(bd[0] - bd[1]), np.float32)
    return wlh, wdh, bdh


def _prep_x(x, nsteps):
    # per core -> [chain2, NW, 32=(mu,d), 8=slot, 256=(p^,j)] fp16
    xs = np.asarray(x[:, :nsteps, :], np.float16)
    outs = []
    for cid in range(NCORES):
        a = xs[cid * BS:(cid + 1) * BS]               # [1024, nsteps, 16]
        a = a.reshape(8, 128, nsteps // 8, 8, 16)     # [m, j, w, s, d]
        a = a.reshape(2, 2, 2, 128, nsteps // 8, 8, 16)  # [c, p^, mu, j, w, s, d]
        a = a.transpose(0, 4, 2, 6, 5, 1, 3)          # [c, w, mu, d, s, p^, j]
        outs.append(np.ascontiguousarray(
            a.reshape(2, nsteps // 8, 32, 8, 256)))
    return outs


_CACHE = {}


def kernel(x, Wi, Wh, b, Wd, bd, nsteps=T, _profile=False):
    from concourse import bass_utils
    wlh, wdh, bdh = _prep_params(Wi, Wh, b, Wd, bd)
    xts = _prep_x(np.asarray(x, np.float32), nsteps)

    if nsteps not in _CACHE:
        _CACHE[nsteps] = _build(nsteps)
    nc = _CACHE[nsteps]

    in_maps = [dict(xt=xts[cid], wlh=wlh, wdh=wdh, bdh=bdh)
               for cid in range(NCORES)]
    res = bass_utils.run_bass_kernel_spmd(nc, in_maps, core_ids=list(range(NCORES)),
                                          trace=_profile)
    outs = [r["out"] for r in res.results]
    full = np.concatenate(outs, 0)
    if _profile:
        return full, res
    return full
